# revision 1
# baseline (speedup 1.0000x reference)
"""Trainium2 Bass kernel for nn_Dist_Conv2D (dist conv with conn-gather + inf-norm).

out[b,o,h,w] = max_j |weights[o,j] - x[b, c_j(o), clamp(h+dh_j(o)), clamp(w+dw_j(o))]| + bias[o]

Strategy (per core; data-parallel over batch, 8 cores x 4 batches):
  - Host precomputes, per batch, 96 row-shifted + W-padded + fully edge-clamped
    planes (c, dh) of x, stored as fp8 e4m3 hi/lo pairs (x = hi + lo), plus a
    constant ones plane. Layout: xp[b, p, sub, 1 + h*66 + k], 66-wide padded
    rows, 1-element margins so dw-shifted flat windows stay in bounds.
  - PE: per output tile (7 rows x 66 padded cols = 462 <= 512 PSUM bank), per
    tap j: 3 accumulating fp8 DoubleRow matmuls (dw = -1/0/+1 via flat window
    offsets; one-hot lhsT selects (c, dh); hi+lo contract via the two
    DoubleRow sub-rows; ones-row adds -w as a hi/lo pair). 0.5 cyc/col.
  - Drains (the bottleneck): two tile flavors balance Act vs DVE:
      A: Act 3-span strip Abs -> A3; DVE tt max -> stream1; stream2 = A3[2].
      B: Act pair strip Abs(P0,P2) -> A02; DVE stt max(P1,A0) -> stream1;
         DVE stt max(-P1,A2) -> stream2.
  - Two bf16 streams DMA'd out; host computes max(S1,S2) + bias in f32.
"""

import sys

if "/opt/trn_rl_repo" not in sys.path:
    sys.path.insert(0, "/opt/trn_rl_repo")

import numpy as np
import ml_dtypes

FP8 = ml_dtypes.float8_e4m3
BF16 = ml_dtypes.bfloat16

import concourse.bass as bass
import concourse.mybir as mybir
from concourse import bacc, bass_utils
from concourse.tile import TileContext

B, C, H, W = 32, 32, 64, 64
OUT_C, CONN_NUM = 128, 3
NCORES = 8
BPC = B // NCORES
WP = W + 2                    # padded row width
RPT = 7                       # rows per full tile (7*66 = 462 <= 512)
NFULL = 9                     # 9 full tiles cover 63 rows; 1 extra row
PLANE = H * WP                # 4224
PLANEB = PLANE + 2            # with 1-elem margins
NP_ = 97                      # 96 (c,dh) planes + ones row
ATILES = (2, 5, 8)            # type-A tiles within each batch (of 0..9)


def _tiles():
    ts = [(t * RPT, RPT) for t in range(NFULL)]
    ts.append((NFULL * RPT, 1))
    return ts


def _build_program():
    nc = bacc.Bacc("TRN2", target_bir_lowering=False, debug=False)
    f32 = mybir.dt.float32
    bf16 = mybir.dt.bfloat16
    fp8 = mybir.dt.float8e4
    Abs = mybir.ActivationFunctionType.Abs
    amax = mybir.AluOpType.max
    amult = mybir.AluOpType.mult
    DR = mybir.MatmulPerfMode.DoubleRow

    xp = nc.dram_tensor("xp", [BPC, NP_, 2, PLANEB], fp8, kind="ExternalInput")
    lh = nc.dram_tensor("lh", [NP_, 3, 3, 2, 128], fp8, kind="ExternalInput")
    y1 = nc.dram_tensor("y1", [BPC, 128, H, W], bf16, kind="ExternalOutput")
    y2 = nc.dram_tensor("y2", [BPC, 128, H, W], bf16, kind="ExternalOutput")

    with TileContext(nc) as tc:
        with (
            tc.tile_pool(name="const", bufs=1) as cpool,
            tc.tile_pool(name="data", bufs=2) as dpool,
            tc.tile_pool(name="work", bufs=3) as wpool,
            tc.tile_pool(name="ps", bufs=2, space="PSUM") as ppool,
        ):
            LH = cpool.tile([NP_, 3, 3, 2, 128], fp8)
            nc.sync.dma_start(LH[:], lh[:])

            for b in range(BPC):
                XP = dpool.tile([NP_, 2, PLANEB], fp8, name="XP", tag="XP")
                nc.sync.dma_start(XP[:], xp[b])

                for t, (h0, rows) in enumerate(_tiles()):
                    L = rows * WP
                    P = ppool.tile([128, 3, 512], f32, name="P", tag="P")
                    for j in range(3):
                        for i, dwi in enumerate((0, 1, 2)):
                            off = 1 + h0 * WP + (dwi - 1)
                            nc.tensor.matmul(
                                P[:, j, 0:L],
                                LH[:, j, dwi],
                                XP[:, :, off : off + L],
                                start=(i == 0),
                                stop=(i == 2),
                                perf_mode=DR,
                            )

                    def strip(ap_3taps, lo, hi_):
                        # [128, k, L] -> [128, k, rows, 64] strip view
                        return ap_3taps.rearrange(
                            "p a (b c) -> p a b c", b=rows
                        )[:, :, :, 1 : 1 + W][:, lo:hi_]

                    S1 = wpool.tile([128, rows, W], bf16, name="S1", tag="S1")
                    P1s = P[:, 1, 0:L].rearrange("p (b c) -> p b c", b=rows)[
                        :, :, 1 : 1 + W
                    ]
                    if t in ATILES:
                        A3 = wpool.tile([128, 3, rows, W], bf16, name="A3", tag="A3")
                        nc.scalar.activation(A3[:], strip(P[:, :, 0:L], 0, 3), Abs)
                        nc.vector.tensor_tensor(S1[:], A3[:, 0], A3[:, 1], amax)
                        S2 = A3[:, 2]
                    else:
                        A02 = wpool.tile([128, 2, rows, W], bf16, name="A02", tag="A02")
                        nc.scalar.activation(
                            A02[:], strip(P[:, 0::2, 0:L], 0, 2), Abs
                        )
                        S2 = wpool.tile([128, rows, W], bf16, name="S2", tag="S2")
                        nc.vector.scalar_tensor_tensor(
                            S1[:], P1s, -3.0e38, A02[:, 0], amax, amax
                        )
                        nc.vector.scalar_tensor_tensor(
                            S2[:], P1s, -1.0, A02[:, 1], amult, amax
                        )
                    nc.sync.dma_start(y1[b, :, h0 : h0 + rows, :], S1[:])
                    nc.sync.dma_start(y2[b, :, h0 : h0 + rows, :], S2[:])
    nc.finalize()
    return nc


def _host_planes(x):
    """x: [B, C, H, W] f32 -> xp [B, NP_, 2, PLANEB] fp8 (hi/lo planes)."""
    n = x.shape[0]
    xw = np.empty((n, C, H, WP), np.float32)
    xw[:, :, :, 1 : 1 + W] = x
    xw[:, :, :, 0] = x[:, :, :, 0]
    xw[:, :, :, WP - 1] = x[:, :, :, W - 1]
    idx = np.arange(H)
    planes = np.empty((n, 3, C, H, WP), np.float32)
    for k, dh in enumerate((-1, 0, 1)):
        planes[:, k] = xw[:, :, np.clip(idx + dh, 0, H - 1), :]
    planes = planes.reshape(n, 96, PLANE)
    hi = planes.astype(FP8)
    lo = (planes - hi.astype(np.float32)).astype(FP8)
    xp = np.zeros((n, NP_, 2, PLANEB), FP8)
    xp[:, 0:96, 0, 1 : 1 + PLANE] = hi
    xp[:, 0:96, 1, 1 : 1 + PLANE] = lo
    xp[:, 96, :, :] = FP8(1.0)
    return xp


def _host_lhs(weights, conn):
    w = np.asarray(weights, np.float32).reshape(OUT_C, CONN_NUM)
    whi = w.astype(FP8).astype(np.float32)
    wlo = (w - whi).astype(FP8).astype(np.float32)
    lh = np.zeros((NP_, 3, 3, 2, 128), np.float32)
    conn = np.asarray(conn).reshape(OUT_C, CONN_NUM)
    for o in range(OUT_C):
        for j in range(CONN_NUM):
            v = int(conn[o, j])
            c, rem = divmod(v, 9)
            kh, kw = divmod(rem, 3)
            dh, dw = kh - 1, kw - 1
            lh[32 * (dh + 1) + c, j, dw + 1, 0, o] = 1.0
            lh[32 * (dh + 1) + c, j, dw + 1, 1, o] = 1.0
            lh[96, j, 1, 0, o] = -whi[o, j]
            lh[96, j, 1, 1, o] = -wlo[o, j]
    return lh.astype(FP8)


_NC_CACHE = []


def kernel(x, weights, bias, conn, _trace=False):
    x = np.asarray(x, np.float32)
    lhs = _host_lhs(weights, conn)
    xp = _host_planes(x)
    if not _NC_CACHE:
        _NC_CACHE.append(_build_program())
    nc = _NC_CACHE[0]
    in_maps = [
        {
            "xp": np.ascontiguousarray(xp[i * BPC : (i + 1) * BPC]),
            "lh": lhs,
        }
        for i in range(NCORES)
    ]
    res = bass_utils.run_bass_kernel_spmd(
        nc, in_maps, core_ids=list(range(NCORES)), trace=_trace
    )
    s1 = np.concatenate(
        [res.results[i]["y1"].astype(np.float32) for i in range(NCORES)], axis=0
    )
    s2 = np.concatenate(
        [res.results[i]["y2"].astype(np.float32) for i in range(NCORES)], axis=0
    )
    out = np.maximum(s1, s2) + np.asarray(bias, np.float32).reshape(1, OUT_C, 1, 1)
    if _trace:
        return out, res
    return out



# revision 3
# speedup vs baseline: 1.1735x; 1.1735x over previous
"""Trainium2 Bass kernel for nn_Dist_Conv2D (dist conv with conn-gather + inf-norm).

out[b,o,h,w] = max_j |weights[o,j] - x[b, c_j(o), clamp(h+dh_j(o)), clamp(w+dw_j(o))]| + bias[o]

Strategy (per core; data-parallel over batch, 8 cores x 4 batches):
  - Host precomputes, per batch, 96 row-shifted + W-padded + fully edge-clamped
    planes (c, dh) of x, stored as fp8 e4m3 hi/lo pairs (x = hi + lo), plus a
    constant ones plane. Layout: xp[b, p, sub, 1 + h*66 + k], 66-wide padded
    rows, 1-element margins so dw-shifted flat windows stay in bounds.
  - PE: per output tile (7 rows x 66 padded cols = 462 <= 512 PSUM bank), per
    tap j: 3 accumulating fp8 DoubleRow matmuls (dw = -1/0/+1 via flat window
    offsets; one-hot lhsT selects (c, dh); hi+lo contract via the two
    DoubleRow sub-rows; ones-row adds -w as a hi/lo pair). 0.5 cyc/col.
  - Drains, two per-tile flavors balancing Act vs DVE (Pool helps via copy):
      R: one DVE tensor_reduce (max, apply_absolute_value) over the tap axis
         -> S1 = max_j |P_j| directly (single stream for these rows).
      A: Act Abs 3-tap strip -> A3; DVE tt max(A0,A1) -> S1; Pool tensor_copy
         A2 -> S2. Host combines max(S1, S2) for A-tile rows.
  - S1 frame [128,64,64] bf16 + packed S2 frame (A-tile rows only): ONE output
    DMA each per batch (HWDGE fixed cost is 625ns/DMA on a single-slot device,
    so DMA instruction count matters as much as bytes).
  - Host computes max(S1, S2[arows]) + bias in f32.
"""

import sys

if "/opt/trn_rl_repo" not in sys.path:
    sys.path.insert(0, "/opt/trn_rl_repo")

import numpy as np
import ml_dtypes

FP8 = ml_dtypes.float8_e4m3
BF16 = ml_dtypes.bfloat16

import concourse.bass as bass
import concourse.mybir as mybir
from concourse import bacc, bass_utils
from concourse.tile import TileContext

B, C, H, W = 32, 32, 64, 64
OUT_C, CONN_NUM = 128, 3
NCORES = 8
BPC = B // NCORES
WP = W + 2                    # padded row width
RPT = 7                       # rows per full tile (7*66 = 462 <= 512 PSUM bank)
NFULL = 9                     # 9 full tiles cover 63 rows; 1 extra row
PLANE = H * WP                # 4224
PLANEB = PLANE + 2            # with 1-elem margins
NP_ = 97                      # 96 (c,dh) planes + ones row

# per-batch tile flavors (10 tiles): 'A' = Act+DVE+Pool two-stream,
# 'R' = single DVE absmax-reduce one-stream. Interleaved for pipelining.
FLAVORS = ("A", "R", "A", "R", "A", "R", "A", "R", "A", "A")


def _tiles():
    ts = [(t * RPT, RPT) for t in range(NFULL)]
    ts.append((NFULL * RPT, 1))
    return ts


def _arows():
    """(h0, rows, s2row0) for each A-flavor tile; S2 frame packs these rows."""
    out = []
    r0 = 0
    for t, (h0, rows) in enumerate(_tiles()):
        if FLAVORS[t] == "A":
            out.append((h0, rows, r0))
            r0 += rows
    return out, r0


AROWS, NA_ROWS = _arows()


def _build_program():
    nc = bacc.Bacc("TRN2", target_bir_lowering=False, debug=False)
    f32 = mybir.dt.float32
    bf16 = mybir.dt.bfloat16
    fp8 = mybir.dt.float8e4
    Abs = mybir.ActivationFunctionType.Abs
    amax = mybir.AluOpType.max
    DR = mybir.MatmulPerfMode.DoubleRow
    X = mybir.AxisListType.X

    xp = nc.dram_tensor("xp", [BPC, NP_, 2, PLANEB], fp8, kind="ExternalInput")
    lh = nc.dram_tensor("lh", [NP_, 3, 3, 2, 128], fp8, kind="ExternalInput")
    y1 = nc.dram_tensor("y1", [BPC, 128, H, W], bf16, kind="ExternalOutput")
    y2 = nc.dram_tensor("y2", [BPC, 128, NA_ROWS, W], bf16, kind="ExternalOutput")

    with TileContext(nc) as tc:
        with (
            tc.tile_pool(name="const", bufs=1) as cpool,
            tc.tile_pool(name="data", bufs=2) as dpool,
            tc.tile_pool(name="work", bufs=3) as wpool,
            tc.tile_pool(name="ps", bufs=2, space="PSUM") as ppool,
        ):
            LH = cpool.tile([NP_, 3, 3, 2, 128], fp8)
            nc.sync.dma_start(LH[:], lh[:])

            for b in range(BPC):
                XP = dpool.tile([NP_, 2, PLANEB], fp8, name="XP", tag="XP")
                nc.sync.dma_start(XP[:], xp[b])
                S1 = dpool.tile([128, H, W], bf16, name="S1", tag="S1")
                S2 = dpool.tile([128, NA_ROWS, W], bf16, name="S2", tag="S2")
                a_idx = 0

                for t, (h0, rows) in enumerate(_tiles()):
                    L = rows * WP
                    P = ppool.tile([128, 3, 512], f32, name="P", tag="P")
                    for j in range(3):
                        for i, dwi in enumerate((0, 1, 2)):
                            off = 1 + h0 * WP + (dwi - 1)
                            nc.tensor.matmul(
                                P[:, j, 0:L],
                                LH[:, j, dwi],
                                XP[:, :, off : off + L],
                                start=(i == 0),
                                stop=(i == 2),
                                perf_mode=DR,
                            )

                    d1 = S1[:, h0 : h0 + rows, :]
                    if FLAVORS[t] == "A":
                        # [128, 3, L] -> [128, 3, rows, 64] strip view
                        strip3 = P[:, :, 0:L].rearrange(
                            "p a (r c) -> p a r c", r=rows
                        )[:, :, :, 1 : 1 + W]
                        A3 = wpool.tile([128, 3, rows, W], bf16, name="A3", tag="A3")
                        nc.scalar.activation(A3[:], strip3, Abs)
                        nc.vector.tensor_tensor(d1, A3[:, 0], A3[:, 1], amax)
                        r0 = AROWS[a_idx][2]
                        a_idx += 1
                        nc.gpsimd.tensor_copy(
                            S2[:, r0 : r0 + rows, :], A3[:, 2]
                        )
                    else:
                        # [128, 3, L] -> [128, rows, 64, 3] (taps innermost)
                        rstrip = P[:, :, 0:L].rearrange(
                            "p a (r c) -> p r c a", r=rows
                        )[:, :, 1 : 1 + W, :]
                        nc.vector.tensor_reduce(
                            d1, rstrip, X, amax, apply_absolute_value=True
                        )

                nc.sync.dma_start(y1[b], S1[:])
                nc.sync.dma_start(y2[b], S2[:])
    nc.finalize()
    return nc


def _host_planes(x):
    """x: [B, C, H, W] f32 -> xp [B, NP_, 2, PLANEB] fp8 (hi/lo planes)."""
    n = x.shape[0]
    xw = np.empty((n, C, H, WP), np.float32)
    xw[:, :, :, 1 : 1 + W] = x
    xw[:, :, :, 0] = x[:, :, :, 0]
    xw[:, :, :, WP - 1] = x[:, :, :, W - 1]
    idx = np.arange(H)
    planes = np.empty((n, 3, C, H, WP), np.float32)
    for k, dh in enumerate((-1, 0, 1)):
        planes[:, k] = xw[:, :, np.clip(idx + dh, 0, H - 1), :]
    planes = planes.reshape(n, 96, PLANE)
    hi = planes.astype(FP8)
    lo = (planes - hi.astype(np.float32)).astype(FP8)
    xp = np.zeros((n, NP_, 2, PLANEB), FP8)
    xp[:, 0:96, 0, 1 : 1 + PLANE] = hi
    xp[:, 0:96, 1, 1 : 1 + PLANE] = lo
    xp[:, 96, :, :] = FP8(1.0)
    return xp


def _host_lhs(weights, conn):
    w = np.asarray(weights, np.float32).reshape(OUT_C, CONN_NUM)
    whi = w.astype(FP8).astype(np.float32)
    wlo = (w - whi).astype(FP8).astype(np.float32)
    lh = np.zeros((NP_, 3, 3, 2, 128), np.float32)
    conn = np.asarray(conn).reshape(OUT_C, CONN_NUM)
    for o in range(OUT_C):
        for j in range(CONN_NUM):
            v = int(conn[o, j])
            c, rem = divmod(v, 9)
            kh, kw = divmod(rem, 3)
            dh, dw = kh - 1, kw - 1
            lh[32 * (dh + 1) + c, j, dw + 1, 0, o] = 1.0
            lh[32 * (dh + 1) + c, j, dw + 1, 1, o] = 1.0
            lh[96, j, 1, 0, o] = -whi[o, j]
            lh[96, j, 1, 1, o] = -wlo[o, j]
    return lh.astype(FP8)


_NC_CACHE = []


def kernel(x, weights, bias, conn, _trace=False):
    x = np.asarray(x, np.float32)
    lhs = _host_lhs(weights, conn)
    xp = _host_planes(x)
    if not _NC_CACHE:
        _NC_CACHE.append(_build_program())
    nc = _NC_CACHE[0]
    in_maps = [
        {
            "xp": np.ascontiguousarray(xp[i * BPC : (i + 1) * BPC]),
            "lh": lhs,
        }
        for i in range(NCORES)
    ]
    res = bass_utils.run_bass_kernel_spmd(
        nc, in_maps, core_ids=list(range(NCORES)), trace=_trace
    )
    s1 = np.concatenate(
        [res.results[i]["y1"].astype(np.float32) for i in range(NCORES)], axis=0
    )
    s2 = np.concatenate(
        [res.results[i]["y2"].astype(np.float32) for i in range(NCORES)], axis=0
    )
    out = s1
    for h0, rows, r0 in AROWS:
        out[:, :, h0 : h0 + rows, :] = np.maximum(
            out[:, :, h0 : h0 + rows, :], s2[:, :, r0 : r0 + rows, :]
        )
    out = out + np.asarray(bias, np.float32).reshape(1, OUT_C, 1, 1)
    if _trace:
        return out, res
    return out
